# revision 1
# baseline (speedup 1.0000x reference)
"""Trainium2 Bass kernel for nn_DecoderRNN (autoregressive LSTM decoder).

Strategy:
  - Pure data parallelism: batch 8192 -> 1024 per core across 8 NeuronCores.
  - Feature-major layout on chip: h^T, c^T are [H=128 partitions, B_local].
    gates^T = W^T blocks (stationary) @ activations (moving), so the
    elementwise LSTM cell update produces h^T directly in the layout the
    next step's matmul needs -- no per-step transposes.
  - The scalar output out_t = W_out @ h_t + b_out feeds the next step's
    input row. That rank-1 contribution is folded into the recurrent
    weights: W~_hh = W_hh + W_ih[:,0:1] @ W_out and
    b~ = b_ih + b_hh + W_ih[:,0] * b_out, which removes the output
    projection from the recurrent critical path entirely. Step 0 uses the
    unfolded W_hh with the observed x value supplied via the z tile.
  - Gate biases ride in the matmul via a constant ones-row appended to the
    z tile (K=65), so PSUM already holds pre-activation gates and the
    ACT engine does pure sigmoid/tanh. ACT is the bottleneck engine here
    (5 transcendental passes over [128,1024] per step).
  - Gates are reordered to [f, i, g, o]: the sigmoid pair (f,i) shares one
    2-bank PSUM tile and evicts in a single [128,1024] ACT op.
  - Matmuls run in float32r (single-pass fp32, ~tf32 precision, 4x faster
    than true fp32 on the PE). State c and all elementwise math stay fp32.
  - The f*c multiply runs on the otherwise-idle GPSIMD engine; the rest of
    the cell update stays on the (faster) DVE.
  - The batch is processed as two 512-column halves so the two dependency
    chains interleave across engines; each half's full evict+cell chain is
    emitted contiguously so the Tile scheduler slots tanh(c) right after
    its inputs instead of behind the other half's evicts.
  - z-side matmuls for step t+1 are pre-issued during step t (they only
    need the prefetched z tile), so after h_t lands only the two h-side
    matmuls gate the next evict.
  - Out rows: step t / half b lands on PSUM partition 32*b + t%32 via
    shifted W_out column blocks, so 64 rows accumulate in one PSUM bank
    and evict once per 32 steps (instead of per-step row copies).
"""

import os
import sys

for _p in ("/opt/trn_rl_repo", "/root/.axon_site/_ro/trn_rl_repo"):
    if os.path.isdir(_p) and _p not in sys.path:
        sys.path.insert(0, _p)

from contextlib import ExitStack

import numpy as np

import concourse.bass as bass  # noqa: F401  (registers types)
import concourse.mybir as mybir
import concourse.tile as tile
from concourse import bacc
from concourse.bass_utils import run_bass_kernel_spmd

NCORES = 8
B, T, F, H, P = 8192, 128, 63, 128, 64
BL = B // NCORES      # 1024 rows per core
I = 2 + F             # 64 LSTM input features + 1 ones-row for bias
G4 = 4 * H            # 512 gate rows
NH = 2                # batch halves (moving-dim chunks of 512)
NW = BL // NH         # 512

_f32 = mybir.dt.float32
_f32r = mybir.dt.float32r

_CACHE: dict = {}


def _build():
    nc = bacc.Bacc("TRN2", target_bir_lowering=False, debug=False)
    AF = mybir.ActivationFunctionType

    zt_d = nc.dram_tensor("zt", [P, I, BL], _f32r, kind="ExternalInput")
    h0_d = nc.dram_tensor("h0t", [H, BL], _f32r, kind="ExternalInput")
    c0_d = nc.dram_tensor("c0t", [H, BL], _f32, kind="ExternalInput")
    # weight layouts: columns are gate rows permuted to [f, i, g, o]
    wz0_d = nc.dram_tensor("wz0t", [I, G4], _f32r, kind="ExternalInput")
    wzf_d = nc.dram_tensor("wzft", [I, G4], _f32r, kind="ExternalInput")
    wh0_d = nc.dram_tensor("whh0t", [H, G4], _f32r, kind="ExternalInput")
    whf_d = nc.dram_tensor("whhft", [H, G4], _f32r, kind="ExternalInput")
    # out-projection weights: one [128,128] buffer with W_out at column 63;
    # slicing 64 columns starting at 63-(32*half + t%32) yields a block with
    # W_out at column 32*half + t%32, so (step t, half b) lands on PSUM
    # partition 32*b + t%32 and 64 rows accumulate in ONE bank, evicted as
    # a single [64, 512] copy per 32 steps.
    wo_d = nc.dram_tensor("woutt", [H, H], _f32r, kind="ExternalInput")
    out_d = nc.dram_tensor("out", [P, BL], _f32, kind="ExternalOutput")

    with ExitStack() as ctx:
        tc = ctx.enter_context(tile.TileContext(nc))
        const = ctx.enter_context(tc.tile_pool(name="const", bufs=1))
        zp = ctx.enter_context(tc.tile_pool(name="z", bufs=4))
        hp = ctx.enter_context(tc.tile_pool(name="h", bufs=2))
        cp = ctx.enter_context(tc.tile_pool(name="c", bufs=2))
        gp = ctx.enter_context(tc.tile_pool(name="g", bufs=3))
        tp = ctx.enter_context(tc.tile_pool(name="t", bufs=3))
        op = ctx.enter_context(tc.tile_pool(name="osb", bufs=3))
        # PSUM budget (8 banks): 2x 2-bank (f,i) tiles + 2x 1-bank g tiles
        # + 1x 1-bank o tile + 1x 1-bank 64-row out-projection accumulator.
        psfi = ctx.enter_context(tc.tile_pool(name="psfi", bufs=2, space="PSUM"))
        psg = ctx.enter_context(tc.tile_pool(name="psg", bufs=2, space="PSUM"))
        pso = ctx.enter_context(tc.tile_pool(name="pso", bufs=1, space="PSUM"))
        pspo = ctx.enter_context(tc.tile_pool(name="pspo", bufs=1, space="PSUM"))

        # step-0-critical tensors first so the pipeline fills ASAP; z0/h0
        # arrive as half-width transfers so half-0 matmuls start sooner
        wz0 = const.tile([I, G4], _f32r, tag="wz0")
        nc.sync.dma_start(wz0[:], wz0_d[:])
        zt0 = zp.tile([I, BL], _f32r, tag="z", name="z0")
        nc.sync.dma_start(zt0[:, 0:NW], zt_d[0, :, 0:NW])
        nc.sync.dma_start(zt0[:, NW:BL], zt_d[0, :, NW:BL])
        h_prev = hp.tile([H, BL], _f32r, tag="h")
        nc.sync.dma_start(h_prev[:, 0:NW], h0_d[:, 0:NW])
        nc.sync.dma_start(h_prev[:, NW:BL], h0_d[:, NW:BL])
        wh0 = const.tile([H, G4], _f32r, tag="wh0")
        nc.sync.dma_start(wh0[:], wh0_d[:])
        c_prev = cp.tile([H, BL], _f32, tag="c")
        nc.sync.dma_start(c_prev[:], c0_d[:])
        wzf = const.tile([I, G4], _f32r, tag="wzf")
        nc.sync.dma_start(wzf[:], wzf_d[:])
        whf = const.tile([H, G4], _f32r, tag="whf")
        nc.sync.dma_start(whf[:], whf_d[:])
        wo = const.tile([H, H], _f32r, tag="wo")
        nc.sync.dma_start(wo[:], wo_d[:])

        def z_mms(t, zt, ps):
            """z-side (and bias) matmul contributions for step t; these only
            need the prefetched z tile, so they are emitted during step t-1
            and run while the PE would otherwise wait for h_{t}."""
            wz = wz0 if t == 0 else wzf
            # step 0 runs while the PE clock ramps: half-size mms shorten the
            # slow first instructions. PSUM start=True is bank-granular, so
            # only the FIRST piece per bank carries start=True.
            nq = 2 if t == 0 else 1
            qw = NW // nq
            for half in range(NH):
                psFI = psfi.tile([H, 2 * NW], _f32, tag="fi", name=f"psFI{t}_{half}")
                psG = psg.tile([H, NW], _f32, tag="g", name=f"psG{t}_{half}")
                psO = pso.tile([H, NW], _f32, tag="o", name=f"psO{t}_{half}")
                ps[(t, half)] = (psFI, psG, psO)
                for j in range(2):
                    for q in range(nq):
                        js = slice(j * NW + q * qw, j * NW + (q + 1) * qw)
                        qs = slice(half * NW + q * qw, half * NW + (q + 1) * qw)
                        nc.tensor.matmul(psFI[:, js], wz[:, j * H : (j + 1) * H],
                                         zt[:, qs], start=(q == 0), stop=False)
                for q in range(nq):
                    qs = slice(half * NW + q * qw, half * NW + (q + 1) * qw)
                    qj = slice(q * qw, (q + 1) * qw)
                    nc.tensor.matmul(psG[:, qj], wz[:, 2 * H : 3 * H], zt[:, qs],
                                     start=(q == 0), stop=False)
                    nc.tensor.matmul(psO[:, qj], wz[:, 3 * H : 4 * H], zt[:, qs],
                                     start=(q == 0), stop=False)

        ps: dict = {}
        z_mms(0, zt0, ps)

        po32: dict = {}  # half -> PSUM tile accumulating 32 out rows

        # out-row groups (start, len): the last group holds only step 63 so
        # its evict+DMA are tiny; the big 31-row group drains during step 63
        _PO_GROUPS = {}
        for _g0, _glen in ((0, 32), (32, 31), (63, 1)):
            for _t in range(_g0, _g0 + _glen):
                _PO_GROUPS[_t] = (_g0, _glen)

        def emit_po(tp_, h_tile):
            """Out row for (step tp_, half b) lands on PSUM partition
            32*b + (tp_-group_start) via shifted W_out column blocks; a
            group's rows accumulate in ONE bank, one evict per group."""
            g0, glen = _PO_GROUPS[tp_]
            j = tp_ - g0
            if j == 0:
                po32[0] = pspo.tile([64, NW], _f32, tag="po32",
                                    name=f"po32_{tp_}")
            for half in range(NH):
                cs = slice(half * NW, (half + 1) * NW)
                blk = 63 - (half * 32 + j)
                nc.tensor.matmul(po32[0][:], wo[:, blk : blk + 64],
                                 h_tile[:, cs],
                                 start=(j == 0 and half == 0),
                                 stop=(j == glen - 1 and half == NH - 1))
            if j == glen - 1:
                orow32 = op.tile([64, NW], _f32, tag="orow", name=f"orow{tp_}")
                nc.vector.tensor_copy(orow32[:], po32[0][:])
                if glen == 1:
                    # single-row group: one strided-source DMA (both halves)
                    nc.sync.dma_start(out_d[g0 : g0 + 1, :],
                                      orow32[0:64:32, :])
                else:
                    for half in range(NH):
                        cs = slice(half * NW, (half + 1) * NW)
                        nc.sync.dma_start(out_d[g0 : g0 + glen, cs],
                                          orow32[32 * half : 32 * half + glen, :])

        prev = None  # (t, h_tile) pending out-projection
        for t in range(P):
            wh = wh0 if t == 0 else whf
            h_new = hp.tile([H, BL], _f32r, tag="h", name=f"h{t}")
            c_new = cp.tile([H, BL], _f32, tag="c", name=f"c{t}")
            # --- PE: h-side mms (critical: feed the ACT evicts), then the
            # delayed out-projection mms for t-1 (ample slack) ---
            for half in range(NH):
                cs = slice(half * NW, (half + 1) * NW)
                psFI, psG, psO = ps[(t, half)]
                for j in range(2):
                    for q in range(2):
                        js = slice(j * NW + q * (NW // 2),
                                   j * NW + (q + 1) * (NW // 2))
                        qs = slice(half * NW + q * (NW // 2),
                                   half * NW + (q + 1) * (NW // 2))
                        nc.tensor.matmul(psFI[:, js], wh[:, j * H : (j + 1) * H],
                                         h_prev[:, qs], start=False, stop=True)
                nc.tensor.matmul(psG[:], wh[:, 2 * H : 3 * H], h_prev[:, cs],
                                 start=False, stop=True)
                nc.tensor.matmul(psO[:], wh[:, 3 * H : 4 * H], h_prev[:, cs],
                                 start=False, stop=True)
            # --- per-half: evicts + full cell chain (scheduler slots ct
            # by priority right after its inputs) ---
            for half in range(NH):
                cs = slice(half * NW, (half + 1) * NW)
                psFI, psG, psO = ps.pop((t, half))
                gFI = gp.tile([H, 2 * NW], _f32, tag="gFI", name=f"gFI{t}_{half}")
                nc.scalar.activation(gFI[:], psFI[:], AF.Sigmoid)
                gG = gp.tile([H, NW], _f32, tag="gG", name=f"gG{t}_{half}")
                nc.scalar.activation(gG[:], psG[:], AF.Tanh)
                gO = gp.tile([H, NW], _f32, tag="gO", name=f"gO{t}_{half}")
                nc.scalar.activation(gO[:], psO[:], AF.Sigmoid)
                f_s = gFI[:, 0:NW]
                i_s = gFI[:, NW : 2 * NW]
                t1 = tp.tile([H, NW], _f32, tag="t1", name=f"t1_{t}_{half}")
                nc.gpsimd.tensor_mul(t1[:], f_s, c_prev[:, cs])
                # quarter-split the whole c chain so tanh(c) and h start as
                # early as possible and the tail hides inside the ACT stream
                for q in range(2):
                    qs = slice(half * NW + q * (NW // 2),
                               half * NW + (q + 1) * (NW // 2))
                    qq = slice(q * (NW // 2), (q + 1) * (NW // 2))
                    t2q = tp.tile([H, NW // 2], _f32, tag=f"t2{q}",
                                  name=f"t2_{t}_{half}_{q}")
                    nc.vector.tensor_mul(t2q[:], i_s[:, qq], gG[:, qq])
                    nc.vector.tensor_add(c_new[:, qs], t1[:, qq], t2q[:])
                    ctq = tp.tile([H, NW // 2], _f32, tag=f"ct{q}",
                                  name=f"ct{t}_{half}_{q}")
                    nc.scalar.activation(ctq[:], c_new[:, qs], AF.Tanh)
                    nc.vector.tensor_mul(h_new[:, qs], gO[:, qq], ctq[:])
            # --- prefetch + pre-issue next step's z work on DMA/PE ---
            if t + 1 < P:
                zt = zp.tile([I, BL], _f32r, tag="z", name=f"z{t + 1}")
                nc.sync.dma_start(zt[:], zt_d[t + 1, :, :])
                z_mms(t + 1, zt, ps)
            # out-projection mms last on PE: a po32 bank-release wait at a
            # 32-step group boundary then can't block critical mms behind it
            if prev is not None:
                emit_po(prev[0], prev[1])
            prev = (t, h_new)
            h_prev, c_prev = h_new, c_new
        # final step's out-projection (closes the second 32-group)
        emit_po(prev[0], prev[1])

    nc.compile()
    return nc


def _get_nc():
    if "nc" not in _CACHE:
        _CACHE["nc"] = _build()
    return _CACHE["nc"]


# gate-row permutation: PyTorch order [i,f,g,o] -> kernel order [f,i,g,o]
_PERM = np.concatenate(
    [np.arange(H, 2 * H), np.arange(0, H), np.arange(2 * H, 3 * H),
     np.arange(3 * H, 4 * H)]
)


def _prep_in_maps(x, z, h0, c0, W_ih, W_hh, b_ih, b_hh, W_out, b_out):
    f = np.float32
    Wihp = W_ih[_PERM]                                   # (512, 64)
    Whhp = W_hh[_PERM]                                   # (512, 128)
    Whfp = Whhp + Wihp[:, 0:1] @ W_out                   # fold out-projection
    b0 = (b_ih + b_hh)[_PERM].astype(f)
    bf = (b0 + Wihp[:, 0] * b_out[0]).astype(f)

    # z-side weights with a trailing bias row (matches the ones-row in zt)
    wz0t = np.concatenate([Wihp.T, b0[None, :]], axis=0).astype(f)   # (65, 512)
    wzft = np.concatenate([Wihp.T, bf[None, :]], axis=0).astype(f)   # (65, 512)
    whh0t = np.ascontiguousarray(Whhp.T, dtype=f)                    # (128, 512)
    whhft = np.ascontiguousarray(Whfp.T, dtype=f)                    # (128, 512)
    # W_out at column 63 of a zeros buffer; emit_po slices 64 columns at a
    # shifted offset so W_out lands on the right PSUM partition
    woutt = np.zeros((H, H), dtype=f)
    woutt[:, 63] = W_out[0]

    in_maps = []
    for m in range(NCORES):
        sl = slice(m * BL, (m + 1) * BL)
        z_aug = np.empty((P, I, BL), dtype=f)
        z_aug[:, 0, :] = 0.0
        z_aug[0, 0, :] = x[sl, -1, 0]
        z_aug[:, 1:-1, :] = np.transpose(z[sl, T - P :, :], (1, 2, 0))
        z_aug[:, -1, :] = 1.0
        in_maps.append(
            {
                "zt": np.ascontiguousarray(z_aug),
                "h0t": np.ascontiguousarray(h0[0, sl, :].T, dtype=f),
                "c0t": np.ascontiguousarray(c0[0, sl, :].T, dtype=f),
                "wz0t": wz0t,
                "wzft": wzft,
                "whh0t": whh0t,
                "whhft": whhft,
                "woutt": woutt,
            }
        )
    return in_maps


def run_on_cores(inputs: dict, **spmd_kwargs):
    """Build + run; returns (full_output, BassKernelResults)."""
    inputs = {k: np.asarray(v, dtype=np.float32) for k, v in inputs.items()}
    nc = _get_nc()
    in_maps = _prep_in_maps(**inputs)
    res = run_bass_kernel_spmd(nc, in_maps, core_ids=list(range(NCORES)), **spmd_kwargs)
    outs = np.concatenate(
        [r["out"].T for r in res.results], axis=0
    )  # (8192, 64)
    outs = outs + np.float32(inputs["b_out"][0])
    return outs[:, :, None].astype(np.float32), res


def kernel(**inputs) -> np.ndarray:
    out, _ = run_on_cores(inputs)
    return out

